# revision 11
# baseline (speedup 1.0000x reference)
"""Trainium2 Bass kernel for 16-head self-attention (B=4, L=2048, D=1024).

Sharding: 8 cores = 4 batches x 2 head-groups (8 heads each). Each core
computes qkv projection, attention and a partial out-projection for its
(batch, head-group); the host sums the two head-group partials per batch.

Per-core pipeline (bf16 matmuls, fp32 PSUM accumulation + softmax chain):
  phase 1: qT/kT in [head_dim_pair, L] layout + v in [L, head] layout
  phase 2: per head-pair c, per i-block: s^T row-packed score MMs ->
           exp on ACT (scale fused; score range makes max-subtraction
           unnecessary) -> v-MM with augmented ones column (M=65)
           accumulating attention out AND softmax denominator in PSUM ->
           reciprocal + DRAM-bounce broadcast -> normalize into outT
  phase 3: y = outT.T @ W_out partial, streamed to DRAM.
"""
import sys

sys.path.insert(0, "/opt/trn_rl_repo")

import numpy as np
import ml_dtypes

import concourse.bacc as bacc
import concourse.mybir as mybir
import concourse.tile as tile
from concourse.bass_utils import run_bass_kernel_spmd

F32 = mybir.dt.float32
BF16 = mybir.dt.bfloat16
EXP = mybir.ActivationFunctionType.Exp

B = 4
L = 2048
D = 1024
HEADS = 16
DH = 64
SCALE = DH ** -0.5
N_CORES = 8
HC = HEADS // 2          # heads per core = 8
NPAIR = HC // 2          # head pairs per core = 4
ND = D // 128            # 8 d-chunks
NL = L // 128            # 16 l-chunks
NI2 = 2                  # i-blocks of 1024
DV = HC * 65             # v_aug width = 520

_CACHE = {}


def _attention(nc, dpool, qt, kt, vt, ot, ppl, npl, pss, pso):
    for c in range(NPAIR):
        for ib in range(4):
            i0 = ib * 512
            isl = slice(i0, i0 + 512)
            o_h = pso.tile([128, 512], F32, tag="o", name="o")
            o_g = pso.tile([128, 512], F32, tag="o", name="o")
            for j in range(NL):
                js = slice(j * 128, (j + 1) * 128)
                sA = pss.tile([128, 512], F32, tag="sA")
                sB = pss.tile([128, 512], F32, tag="sB")
                nc.tensor.matmul(sA[:], kt[c][0:64, js], qt[c][0:64, isl],
                                 start=True, stop=True)
                nc.tensor.matmul(sB[:], kt[c][64:128, js], qt[c][64:128, isl],
                                 start=True, stop=True)
                pA = ppl.tile([128, 512], BF16, tag="pA")
                pB = ppl.tile([128, 512], BF16, tag="pB")
                nc.scalar.activation(pA[:], sA[:], EXP, scale=float(SCALE))
                nc.scalar.activation(pB[:], sB[:], EXP, scale=float(SCALE))
                st, sp = j == 0, j == NL - 1
                va = vt[j][:, c * 130:c * 130 + 65]
                vb = vt[j][:, c * 130 + 65:c * 130 + 130]
                nc.tensor.matmul(o_h[0:65, :], va, pA[:], start=st, stop=sp)
                nc.tensor.matmul(o_g[0:65, :], vb, pB[:], start=st, stop=sp)
            # normalize: o rows 0-63 / denom row 64
            for k, o_t in enumerate((o_h, o_g)):
                row = slice(0, 64) if k == 0 else slice(64, 128)
                dcp = npl.tile([1, 512], F32, tag="dcp")
                rcp = npl.tile([1, 512], F32, tag="rcp")
                rb = npl.tile([64, 512], F32, tag="rb")
                nc.vector.tensor_copy(dcp[:], o_t[64:65, :])
                nc.vector.reciprocal_approx_fast(out=rcp[:], in_=dcp[:])
                dst = dpool.tile([1, 512], F32, tag="rst")
                nc.sync.dma_start(dst[:], rcp[:])
                nc.sync.dma_start(rb[:], dst[:].to_broadcast([64, 512]))
                nc.vector.tensor_mul(ot[c][row, isl], o_t[0:64, :], rb[:])


def _build():
    nc = bacc.Bacc("TRN2", target_bir_lowering=False)

    xT_d = nc.dram_tensor("xT", [D, L], BF16, kind="ExternalInput")
    wqk_d = nc.dram_tensor("wqk", [D, 1024], BF16, kind="ExternalInput")
    wv_d = nc.dram_tensor("wv", [D, 512], BF16, kind="ExternalInput")
    wout_d = nc.dram_tensor("wout", [512, 1024], BF16, kind="ExternalInput")
    bqk_d = nc.dram_tensor("bqk", [8, 128, 1], F32, kind="ExternalInput")
    y_d = nc.dram_tensor("y", [L, D], F32, kind="ExternalOutput")

    with tile.TileContext(nc) as tc:
        with (
            tc.tile_pool(name="persist", bufs=1) as pp,
            tc.tile_pool(name="dstage", bufs=8, space="DRAM") as dpool,
        ):
            # persistent tiles
            qt = [pp.tile([128, L], BF16, tag=f"qt{c}", name=f"qt{c}")
                  for c in range(NPAIR)]
            kt = [pp.tile([128, L], BF16, tag=f"kt{c}", name=f"kt{c}")
                  for c in range(NPAIR)]
            vt = [pp.tile([128, DV], BF16, tag=f"v{l}", name=f"v{l}")
                  for l in range(NL)]
            wout = [pp.tile([128, 1024], BF16, tag=f"wo{c}", name=f"wo{c}")
                    for c in range(NPAIR)]
            bias = [pp.tile([128, 1], F32, tag=f"b{t}", name=f"b{t}")
                    for t in range(8)]
            ones_f = pp.tile([128, 1], F32, tag="ones")

            nc.vector.memset(ones_f[:], 1.0)
            for t in range(8):
                nc.sync.dma_start(bias[t][:], bqk_d[t])

            # ---------------- phase 1: qkv projection ----------------
            with (
                tc.tile_pool(name="ph1", bufs=1) as p1,
                tc.tile_pool(name="wstream", bufs=4) as wsp,
                tc.tile_pool(name="acc1", bufs=8, space="PSUM") as acc1,
            ):
                xt = [p1.tile([128, L], BF16, tag=f"xt{d}", name=f"xt{d}")
                      for d in range(ND)]
                wv = [p1.tile([128, 512], BF16, tag=f"wv{d}", name=f"wv{d}")
                      for d in range(ND)]
                for d in range(ND):
                    nc.sync.dma_start(
                        xt[d][:], xT_d[d * 128:(d + 1) * 128, :])
                for d in range(ND):
                    nc.sync.dma_start(
                        wv[d][:], wv_d[d * 128:(d + 1) * 128, :])

                # qT (t=0..3) and kT (t=4..7) chunks: [128, L] each
                for t in range(8):
                    dst = qt[t] if t < 4 else kt[t - 4]
                    wtiles = []
                    for d in range(ND):
                        w = wsp.tile([128, 128], BF16, tag="wqk", name="wqk")
                        nc.sync.dma_start(
                            w[:],
                            wqk_d[d * 128:(d + 1) * 128,
                                  t * 128:(t + 1) * 128])
                        wtiles.append(w)
                    psums = [acc1.tile([128, 512], F32, tag="acc", name="acc")
                             for _ in range(4)]
                    for d in range(ND):
                        for n in range(4):
                            nc.tensor.matmul(
                                psums[n][:], wtiles[d][:],
                                xt[d][:, n * 512:(n + 1) * 512],
                                start=(d == 0), stop=(d == ND - 1))
                    for n in range(4):
                        nc.vector.tensor_scalar_add(
                            dst[:, n * 512:(n + 1) * 512], psums[n][:], bias[t][:])

                # v natural layout with per-head ones column (65-stride)
                for l in range(NL):
                    ps = acc1.tile([128, 512], F32, tag="acc")
                    for d in range(ND):
                        nc.tensor.matmul(
                            ps[:], xt[d][:, l * 128:(l + 1) * 128], wv[d][:],
                            start=(d == 0), stop=(d == ND - 1))
                    v3 = vt[l][:].rearrange("p (h w) -> p h w", w=65)
                    nc.vector.tensor_copy(
                        v3[:, :, 0:64],
                        ps[:].rearrange("p (h w) -> p h w", w=64))
                    nc.vector.tensor_copy(
                        v3[:, :, 64:65],
                        ones_f[:, None, :].broadcast_to([128, HC, 1]))

                for c in range(NPAIR):
                    nc.sync.dma_start(
                        wout[c][:],
                        wout_d[c * 128:(c + 1) * 128, :])

            # ---------------- phase 2: attention ----------------
            with (
                tc.tile_pool(name="ph2", bufs=1) as p2,
                tc.tile_pool(name="ppool", bufs=2) as ppl,
                tc.tile_pool(name="npool", bufs=4) as npl,
            ):
                ot = [p2.tile([128, L], BF16, tag=f"ot{c}", name=f"ot{c}")
                      for c in range(NPAIR)]
                with (
                    tc.tile_pool(name="ps_s", bufs=2, space="PSUM") as pss,
                    tc.tile_pool(name="ps_o", bufs=4, space="PSUM") as pso,
                ):
                    _attention(nc, dpool, qt, kt, vt, ot, ppl, npl, pss, pso)

                # ---------------- phase 3: out projection ----------------
                with (
                    tc.tile_pool(name="ystage", bufs=3) as ysp,
                    tc.tile_pool(name="ps_y", bufs=4, space="PSUM") as psy,
                ):
                    for i in range(NL):
                        psm = [psy.tile([128, 512], F32, tag="y", name="y")
                               for _ in range(2)]
                        for c in range(NPAIR):
                            for m in range(2):
                                nc.tensor.matmul(
                                    psm[m][:], ot[c][:, i * 128:(i + 1) * 128],
                                    wout[c][:, m * 512:(m + 1) * 512],
                                    start=(c == 0), stop=(c == NPAIR - 1))
                        for m in range(2):
                            yst = ysp.tile([128, 512], F32, tag="yst", name="yst")
                            nc.vector.tensor_copy(yst[:], psm[m][:])
                            nc.sync.dma_start(
                                y_d[i * 128:(i + 1) * 128,
                                    m * 512:(m + 1) * 512], yst[:])

    nc.finalize()
    return nc


def _get_nc():
    if "nc" not in _CACHE:
        _CACHE["nc"] = _build()
    return _CACHE["nc"]


def _make_in_maps(x, W_qkv, b_qkv, W_out):
    xT = [np.ascontiguousarray(x[b].T).astype(ml_dtypes.bfloat16)
          for b in range(B)]
    in_maps = []
    for b in range(B):
        for g in range(2):
            sl = slice(g * 512, (g + 1) * 512)
            wqk_c = np.ascontiguousarray(
                np.concatenate([W_qkv[:, sl],
                                W_qkv[:, 1024 + g * 512:1024 + (g + 1) * 512]],
                               axis=1)).astype(ml_dtypes.bfloat16)
            wv_c = np.ascontiguousarray(
                W_qkv[:, 2048 + g * 512:2048 + (g + 1) * 512]).astype(
                    ml_dtypes.bfloat16)
            wout_c = np.ascontiguousarray(W_out[sl, :]).astype(
                ml_dtypes.bfloat16)
            bqk_c = np.concatenate(
                [b_qkv[g * 512:(g + 1) * 512],
                 b_qkv[1024 + g * 512:1024 + (g + 1) * 512]]).reshape(8, 128, 1)
            in_maps.append({
                "xT": xT[b],
                "wqk": wqk_c,
                "wv": wv_c,
                "wout": wout_c,
                "bqk": np.ascontiguousarray(bqk_c),
            })
    return in_maps


def kernel(x, W_qkv, b_qkv, W_out, b_out):
    x = np.asarray(x, dtype=np.float32)
    W_qkv = np.asarray(W_qkv, dtype=np.float32)
    b_qkv = np.asarray(b_qkv, dtype=np.float32)
    W_out = np.asarray(W_out, dtype=np.float32)
    b_out = np.asarray(b_out, dtype=np.float32)

    nc = _get_nc()
    in_maps = _make_in_maps(x, W_qkv, b_qkv, W_out)
    res = run_bass_kernel_spmd(nc, in_maps, core_ids=list(range(N_CORES)))

    # v-bias flows additively through softmax (rows sum to 1): + b_v @ W_out
    y_bias = b_qkv[2048:3072] @ W_out + b_out
    out = np.empty((B, L, D), dtype=np.float32)
    for b in range(B):
        out[b] = res.results[2 * b]["y"] + res.results[2 * b + 1]["y"] + y_bias
    return out
